# revision 1
# baseline (speedup 1.0000x reference)
"""2-layer GIN (mean aggregation) on 8 Trainium2 NeuronCores.

Strategy (graph/data parallel, per the node-sharding hint):
  - Nodes sharded 8 ways by dst; each core owns its node shard and all
    edges whose dst lands in the shard.
  - Per layer, per core: agg_T[f, d] = x_T + sum_e msg[src(e)] * (1/deg),
    then the dense epilogue out_T = W.T @ agg_T + b (+relu), feature-major.
  - Edge messages are fetched with gpsimd.dma_gather (SWDGE) straight from
    an HBM copy of the full feature table (fp16). int16 gather indices cap
    a table at 32768 rows, so the node table is split into 4 banks of
    25000 and each core's edge list is sorted by (bank, dst).
  - Segment-sum: per 128-edge chunk, DVE builds a full-width fp16 one-hot
    [128 e, 256 d] = (iota == dstrel) * deginv against host-baked
    window-relative dst offsets; PE accumulates
    psum[f, 0:256] += msgs[e, f].T @ onehot over a 4-chunk (512-edge)
    group; the group psum is drain-added into the SBUF agg accumulator at
    a register-driven (data-dependent window base) offset. The SPMD
    instruction stream is identical across cores — only table data varies.
  - The two GIN layers are two executions of the same NEFF; the host
    concatenates the per-core shard outputs between layers (the halo
    exchange runs through host memory).
"""

import numpy as np

import concourse.bass as bass
import concourse.mybir as mybir
import concourse.tile as tile
from concourse import bacc
from concourse.bass_utils import run_bass_kernel_spmd

F32 = mybir.dt.float32
F16 = mybir.dt.float16
I32 = mybir.dt.int32
I16 = mybir.dt.int16

FULL_CFG = dict(
    n_nodes=100000,   # global nodes (gather table rows)
    n_cores=8,
    npc=12500,        # nodes per core
    npad=12800,       # padded nodes per core (multiple of 512)
    nbank=4,
    bank_sz=25000,
    e_bank=51200,     # padded edges per (core, bank); multiple of call_e
    call_e=1024,      # edges per dma_gather call (SWDGE ring fits 1024)
)

D = 128
CHUNK = 128
GROUP_CHUNKS = 4   # 512-edge psum accumulation group
PSW = 256          # psum window width (dst columns per group)
SENT_DSTREL = PSW - 1   # sentinel: in-window column, dege=0 -> contributes 0


# ----------------------------------------------------------------------------
# device kernel
# ----------------------------------------------------------------------------

def build_kernel(cfg):
    nbank, bank_sz = cfg["nbank"], cfg["bank_sz"]
    e_bank, call_e, npad = cfg["e_bank"], cfg["call_e"], cfg["npad"]
    n_nodes = cfg["n_nodes"]
    calls_pb = e_bank // call_e
    chunks_pc = call_e // CHUNK
    groups_pc = chunks_pc // GROUP_CHUNKS
    chunks_pb = e_bank // CHUNK
    groups_pb = chunks_pb // GROUP_CHUNKS
    n_groups = nbank * groups_pb
    n_slabs = npad // 512

    nc = bacc.Bacc("TRN2", target_bir_lowering=False, debug=False,
                   num_devices=cfg["n_cores"])

    xfull = nc.dram_tensor("xfull", [n_nodes, D], F16, kind="ExternalInput")
    xT = nc.dram_tensor("xT", [D, npad], F32, kind="ExternalInput")
    w = nc.dram_tensor("w", [D, D], F32, kind="ExternalInput")
    bcol = nc.dram_tensor("bcol", [D, 1], F32, kind="ExternalInput")
    relu_lo = nc.dram_tensor("relu_lo", [D, 1], F32, kind="ExternalInput")
    idx = nc.dram_tensor("idx", [nbank, 128, e_bank // 16], I16, kind="ExternalInput")
    dstrel = nc.dram_tensor("dstrel", [nbank, 128, chunks_pb], F32, kind="ExternalInput")
    dege = nc.dram_tensor("dege", [nbank, 128, chunks_pb], F32, kind="ExternalInput")
    wbase = nc.dram_tensor("wbase", [1, n_groups], I32, kind="ExternalInput")
    outT = nc.dram_tensor("outT", [D, npad], F32, kind="ExternalOutput")

    with tile.TileContext(nc) as tc:
        with (
            tc.tile_pool(name="const", bufs=1) as cpool,
            tc.tile_pool(name="agg", bufs=1) as apool,
            tc.tile_pool(name="btab", bufs=2) as bpool,
            tc.tile_pool(name="msgs", bufs=3) as mpool,
            tc.tile_pool(name="oh", bufs=4) as opool,
            tc.tile_pool(name="acc", bufs=3, space="PSUM") as pspool,
            tc.tile_pool(name="dense", bufs=2, space="PSUM") as dpool,
            tc.tile_pool(name="osb", bufs=2) as spool,
        ):
            iota_i = cpool.tile([128, PSW], I32)
            nc.gpsimd.iota(iota_i[:], pattern=[[1, PSW]], channel_multiplier=0)
            iota_h = cpool.tile([128, PSW], F16)
            nc.vector.tensor_copy(iota_h[:], iota_i[:])

            w_sb = cpool.tile([D, D], F32)
            nc.sync.dma_start(w_sb[:], w[:])
            b_sb = cpool.tile([D, 1], F32)
            nc.sync.dma_start(b_sb[:], bcol[:])
            rl_sb = cpool.tile([D, 1], F32)
            nc.sync.dma_start(rl_sb[:], relu_lo[:])
            wbase_sb = cpool.tile([1, n_groups], I32)
            nc.sync.dma_start(wbase_sb[:], wbase[:])

            agg = apool.tile([128, npad], F32)
            nc.sync.dma_start(agg[:], xT[:])

            for bb in range(nbank):
                idx_b = bpool.tile([128, e_bank // 16], I16, tag="idx")
                nc.sync.dma_start(idx_b[:], idx[bb])
                dr_b = bpool.tile([128, chunks_pb], F32, tag="dr")
                nc.sync.dma_start(dr_b[:], dstrel[bb])
                dg_b = bpool.tile([128, chunks_pb], F32, tag="dg")
                nc.sync.dma_start(dg_b[:], dege[bb])
                src_rows = xfull[bb * bank_sz:(bb + 1) * bank_sz, :]

                for call in range(calls_pb):
                    msgs = mpool.tile([128, chunks_pc, D], F16, tag="msgs")
                    nc.gpsimd.dma_gather(
                        msgs[:], src_rows,
                        idx_b[:, call * (call_e // 16):(call + 1) * (call_e // 16)],
                        call_e, call_e, D,
                    )
                    for g2 in range(groups_pc):
                        gidx = bb * groups_pb + call * groups_pc + g2
                        acc = pspool.tile([128, PSW], F32, tag="acc")
                        for j4 in range(GROUP_CHUNKS):
                            jc = g2 * GROUP_CHUNKS + j4
                            ci = call * chunks_pc + jc
                            oh = opool.tile([128, PSW], F16, tag="oh")
                            nc.vector.tensor_scalar(
                                oh[:], iota_h[:],
                                dr_b[:, ci:ci + 1], dg_b[:, ci:ci + 1],
                                mybir.AluOpType.is_equal, mybir.AluOpType.mult,
                            )
                            nc.tensor.matmul(
                                acc[:], msgs[:, jc, :], oh[:],
                                start=(j4 == 0), stop=(j4 == GROUP_CHUNKS - 1),
                            )
                        dregs = nc.alloc_registers(engines=(mybir.EngineType.DVE,))
                        nc.reg_load(dregs, wbase_sb[0:1, gidx:gidx + 1])
                        sw = nc.snap(dregs, donate=True, min_val=0,
                                     max_val=npad - PSW)
                        nc.vector.tensor_tensor(
                            agg[:, bass.ds(sw, PSW)], acc[:],
                            agg[:, bass.ds(sw, PSW)], mybir.AluOpType.add,
                        )

            for s in range(n_slabs):
                dop = dpool.tile([128, 512], F32, tag="dop")
                nc.tensor.matmul(dop[:], w_sb[:], agg[:, s * 512:(s + 1) * 512],
                                 start=True, stop=True)
                ot = spool.tile([128, 512], F32, tag="ot")
                nc.vector.tensor_scalar(
                    ot[:], dop[:], b_sb[:, 0:1], rl_sb[:, 0:1],
                    mybir.AluOpType.add, mybir.AluOpType.max,
                )
                nc.sync.dma_start(outT[:, s * 512:(s + 1) * 512], ot[:])

    nc.compile()
    return nc


# ----------------------------------------------------------------------------
# host-side graph preprocessing
# ----------------------------------------------------------------------------

def prep_tables(cfg, src, dst):
    """Per-core gather/scatter tables. Returns a list of dicts (one per core)."""
    n_nodes, n_cores, npc = cfg["n_nodes"], cfg["n_cores"], cfg["npc"]
    nbank, bank_sz = cfg["nbank"], cfg["bank_sz"]
    e_bank, call_e, npad = cfg["e_bank"], cfg["call_e"], cfg["npad"]
    chunks_pb = e_bank // CHUNK
    groups_pb = chunks_pb // GROUP_CHUNKS
    n_groups = nbank * groups_pb

    deg = np.bincount(dst, minlength=n_nodes)
    deginv = (1.0 / np.maximum(deg, 1)).astype(np.float32)

    core_of = dst // npc
    out = []
    for c in range(n_cores):
        m = core_of == c
        s_c = src[m]
        dl_c = (dst[m] - c * npc).astype(np.int64)
        dg_c = deginv[dst[m]]
        b_c = s_c // bank_sz
        sl_c = (s_c - b_c * bank_sz).astype(np.int16)
        order = np.lexsort((dl_c, b_c))
        s_o, dl_o, dg_o, b_o = sl_c[order], dl_c[order], dg_c[order], b_c[order]

        idx_t = np.zeros((nbank, 128, e_bank // 16), np.int16)
        dr_t = np.full((nbank, 128, chunks_pb), SENT_DSTREL, np.float32)
        dg_t = np.zeros((nbank, 128, chunks_pb), np.float32)
        wbase_t = np.zeros((1, n_groups), np.int32)

        for bb in range(nbank):
            sel = b_o == bb
            sl_b, dl_b, dg_b = s_o[sel], dl_o[sel], dg_o[sel]
            n = len(sl_b)
            assert n <= e_bank, f"core {c} bank {bb}: {n} > e_bank {e_bank}"
            sl_p = np.zeros(e_bank, np.int16)
            sl_p[:n] = sl_b
            dl_p = np.full(e_bank, -1, np.int64)
            dl_p[:n] = dl_b
            dg_p = np.zeros(e_bank, np.float32)
            dg_p[:n] = dg_b

            dl_gr = dl_p.reshape(groups_pb, GROUP_CHUNKS * CHUNK)
            g_real = dl_gr[:, 0] >= 0
            g_first = np.where(g_real, dl_gr[:, 0], 0)
            wb = np.minimum(g_first, npad - PSW)
            g_max = dl_gr.max(axis=1)
            assert (g_max[g_real] - wb[g_real] < PSW).all(), \
                f"core {c} bank {bb}: group span exceeds {PSW}"

            wb_e = np.repeat(wb, GROUP_CHUNKS * CHUNK)
            rel = dl_p - wb_e
            real_e = dl_p >= 0
            dr_vals = np.where(real_e, rel, SENT_DSTREL).astype(np.float32)

            dr_t[bb] = dr_vals.reshape(chunks_pb, CHUNK).T
            dg_t[bb] = dg_p.reshape(chunks_pb, CHUNK).T
            wbase_t[0, bb * groups_pb:(bb + 1) * groups_pb] = wb

            # idx wrapping: within each call, idx i -> [16g + i%16, i//16]
            a = sl_p.reshape(e_bank // call_e, call_e // 16, 16)
            blocks = [np.tile(a[k].T, (8, 1)) for k in range(e_bank // call_e)]
            idx_t[bb] = np.concatenate(blocks, axis=1)

        out.append(dict(idx=idx_t, dstrel=dr_t, dege=dg_t, wbase=wbase_t))
    return out


# ----------------------------------------------------------------------------
# full forward
# ----------------------------------------------------------------------------

_compiled = {}


def _get_kernel(cfg):
    key = tuple(sorted(cfg.items()))
    if key not in _compiled:
        _compiled[key] = build_kernel(cfg)
    return _compiled[key]


def run_layer(cfg, nc, tables, x_full, w, b, relu):
    n_cores, npc, npad = cfg["n_cores"], cfg["npc"], cfg["npad"]
    x16 = np.ascontiguousarray(x_full, np.float16)
    w = np.ascontiguousarray(w, np.float32)
    bcol = np.ascontiguousarray(b, np.float32).reshape(D, 1)
    rl = np.full((D, 1), 0.0 if relu else -3.4e38, np.float32)
    in_maps = []
    for c in range(n_cores):
        xT = np.zeros((D, npad), np.float32)
        xT[:, :npc] = x_full[c * npc:(c + 1) * npc].T
        t = tables[c]
        in_maps.append({
            "xfull": x16, "xT": xT, "w": w, "bcol": bcol, "relu_lo": rl,
            "idx": t["idx"], "dstrel": t["dstrel"], "dege": t["dege"],
            "wbase": t["wbase"],
        })
    res = run_bass_kernel_spmd(nc, in_maps, core_ids=list(range(n_cores)))
    out = np.empty((n_cores * npc, D), np.float32)
    for c in range(n_cores):
        out[c * npc:(c + 1) * npc] = res.results[c]["outT"][:, :npc].T
    return out


def gin_forward(cfg, in_feat, src, dst, W1, b1, W2, b2):
    nc = _get_kernel(cfg)
    tables = prep_tables(cfg, src, dst)
    x = np.ascontiguousarray(in_feat, np.float32)
    h = run_layer(cfg, nc, tables, x, W1, b1, relu=True)
    return run_layer(cfg, nc, tables, h, W2, b2, relu=False)


def kernel(in_feat, src, dst, W1, b1, W2, b2):
    in_feat = np.asarray(in_feat, np.float32)
    src = np.asarray(src, np.int64)
    dst = np.asarray(dst, np.int64)
    W1 = np.asarray(W1, np.float32)
    b1 = np.asarray(b1, np.float32)
    W2 = np.asarray(W2, np.float32)
    b2 = np.asarray(b2, np.float32)
    return gin_forward(FULL_CFG, in_feat, src, dst, W1, b1, W2, b2)



# revision 7
# speedup vs baseline: 1.0799x; 1.0799x over previous
"""2-layer GIN (mean aggregation) on 8 Trainium2 NeuronCores.

Strategy (graph/data parallel, per the node-sharding hint):
  - Nodes sharded 8 ways by dst; each core owns its node shard and all
    edges whose dst lands in the shard.
  - Per layer, per core: agg_T[f, d] = x_T + sum_e msg[src(e)] * (1/deg),
    then the dense epilogue out_T = W.T @ agg_T + b (+relu), feature-major.
  - Edge messages are fetched with gpsimd.dma_gather (SWDGE) straight from
    an HBM copy of the full feature table (fp16). int16 gather indices cap
    a table at 32768 rows, so the node table is split into 4 banks of
    25000 and each core's edge list is sorted by (bank, dst).
  - Segment-sum: per 128-edge chunk, DVE builds a [128 e, 192 d] fp16
    one-hot = (iota == dstrel) * deginv against host-baked window-relative
    dst offsets; PE accumulates psum[f, 0:192] += msgs[e, f].T @ onehot
    over a 4-chunk (512-edge) group; the group psum is drain-added into
    the fp16 SBUF agg accumulator at a register-driven (data-dependent
    window base) offset, alternating between the DVE and Pool engines so
    neither becomes the bottleneck. The agg tile is DMA-initialized with
    x_T so the (1+eps)*x term needs no extra pass.
  - Gather calls are 5120 edges each (large calls amortize the SWDGE
    fixed descriptor-generation overhead on the Pool engine) and are
    emitted one call ahead of their compute so the DMA engines stay busy.
  - Dense epilogue: PE matmul (fp16) per 512-column slab, then the
    Activation engine applies bias+relu (or bias only) straight out of
    PSUM into an fp16 staging tile that is DMA'd out.
  - The two GIN layers are two executions of near-identical NEFFs (they
    differ only in relu-vs-identity); the host concatenates the per-core
    shard outputs between layers (the halo exchange runs through host
    memory). The SPMD instruction stream is identical across cores —
    only table data varies.
"""

import numpy as np

import concourse.bass as bass
import concourse.mybir as mybir
import concourse.tile as tile
from concourse import bacc
from concourse.bass_utils import run_bass_kernel_spmd

F32 = mybir.dt.float32
F16 = mybir.dt.float16
I32 = mybir.dt.int32
I16 = mybir.dt.int16

FULL_CFG = dict(
    n_nodes=100000,   # global nodes (gather table rows)
    n_cores=8,
    npc=12500,        # nodes per core
    npad=12800,       # padded nodes per core (multiple of 512)
    nbank=4,
    bank_sz=25000,
    e_bank=51200,     # padded edges per (core, bank); multiple of call_e
    call_e=1024,      # edges per dma_gather call
)

D = 128
CHUNK = 128
GROUP_CHUNKS = 4   # 512-edge psum accumulation group
PSW = 192          # psum window width (dst columns per group)
SENT_DSTREL = PSW - 1   # sentinel: in-window column, dege=0 -> contributes 0


# ----------------------------------------------------------------------------
# device kernel
# ----------------------------------------------------------------------------

def build_kernel(cfg, relu):
    nbank, bank_sz = cfg["nbank"], cfg["bank_sz"]
    e_bank, call_e, npad = cfg["e_bank"], cfg["call_e"], cfg["npad"]
    calls_pb = e_bank // call_e
    chunks_pc = call_e // CHUNK
    groups_pc = chunks_pc // GROUP_CHUNKS
    chunks_pb = e_bank // CHUNK
    groups_pb = chunks_pb // GROUP_CHUNKS
    n_groups = nbank * groups_pb
    n_slabs = npad // 512

    nc = bacc.Bacc("TRN2", target_bir_lowering=False, debug=False,
                   num_devices=cfg["n_cores"],
                   dynamic_dma_scratch_size=98304)

    xfull = nc.dram_tensor("xfull", [cfg["n_nodes"], D], F16, kind="ExternalInput")
    xT = nc.dram_tensor("xT", [D, npad], F16, kind="ExternalInput")
    w = nc.dram_tensor("w", [D, D], F16, kind="ExternalInput")
    bcol = nc.dram_tensor("bcol", [D, 1], F32, kind="ExternalInput")
    idx = nc.dram_tensor("idx", [nbank, 128, e_bank // 16], I16, kind="ExternalInput")
    dstrel = nc.dram_tensor("dstrel", [nbank, 128, chunks_pb], F32, kind="ExternalInput")
    dege = nc.dram_tensor("dege", [nbank, 128, chunks_pb], F32, kind="ExternalInput")
    wbase = nc.dram_tensor("wbase", [1, n_groups], I32, kind="ExternalInput")
    outT = nc.dram_tensor("outT", [D, npad], F16, kind="ExternalOutput")

    act_func = (mybir.ActivationFunctionType.Relu if relu
                else mybir.ActivationFunctionType.Identity)

    with tile.TileContext(nc) as tc:
        with (
            tc.tile_pool(name="const", bufs=1) as cpool,
            tc.tile_pool(name="agg", bufs=1) as apool,
            tc.tile_pool(name="btab", bufs=2) as bpool,
            tc.tile_pool(name="msgs", bufs=3) as mpool,
            tc.tile_pool(name="oh", bufs=4) as opool,
            tc.tile_pool(name="dtmp", bufs=3) as tpool,
            tc.tile_pool(name="acc", bufs=3, space="PSUM") as pspool,
            tc.tile_pool(name="dense", bufs=2, space="PSUM") as dpool,
            tc.tile_pool(name="osb", bufs=2) as spool,
        ):
            iota_i = cpool.tile([128, PSW], I32)
            nc.gpsimd.iota(iota_i[:], pattern=[[1, PSW]], channel_multiplier=0)
            iota_h = cpool.tile([128, PSW], F16)
            nc.vector.tensor_copy(iota_h[:], iota_i[:])

            w_sb = cpool.tile([D, D], F16)
            nc.sync.dma_start(w_sb[:], w[:])
            b_sb = cpool.tile([D, 1], F32)
            nc.sync.dma_start(b_sb[:], bcol[:])
            wbase_sb = cpool.tile([1, n_groups], I32)
            nc.sync.dma_start(wbase_sb[:], wbase[:])

            # agg starts as x_T; group psums are drain-added into it.
            agg = apool.tile([128, npad], F16)
            nc.sync.dma_start(agg[:], xT[:])

            # (bank, call) schedule, with table loads and gathers emitted one
            # call ahead of their compute so desc-gen/DMA overlap compute.
            sched = [(bb, call) for bb in range(nbank) for call in range(calls_pb)]
            btabs = {}
            mtiles = [None] * len(sched)

            def load_bank(bb):
                idx_b = bpool.tile([128, e_bank // 16], I16, tag="idx")
                nc.sync.dma_start(idx_b[:], idx[bb])
                dr_b = bpool.tile([128, chunks_pb], F32, tag="dr")
                nc.sync.dma_start(dr_b[:], dstrel[bb])
                dg_b = bpool.tile([128, chunks_pb], F32, tag="dg")
                nc.sync.dma_start(dg_b[:], dege[bb])
                btabs[bb] = (idx_b, dr_b, dg_b)

            def emit_gather(k):
                bb, call = sched[k]
                if call == 0:
                    load_bank(bb)
                idx_b = btabs[bb][0]
                msgs = mpool.tile([128, chunks_pc, D], F16, tag="msgs")
                nc.gpsimd.dma_gather(
                    msgs[:], xfull[bb * bank_sz:(bb + 1) * bank_sz, :],
                    idx_b[:, call * (call_e // 16):(call + 1) * (call_e // 16)],
                    call_e, call_e, D,
                )
                mtiles[k] = msgs

            emit_gather(0)
            for k, (bb, call) in enumerate(sched):
                if k + 1 < len(sched):
                    emit_gather(k + 1)
                msgs = mtiles[k]
                _, dr_b, dg_b = btabs[bb]
                for g2 in range(groups_pc):
                    gidx = bb * groups_pb + call * groups_pc + g2
                    acc = pspool.tile([128, PSW], F32, tag="acc")
                    for j4 in range(GROUP_CHUNKS):
                        jc = g2 * GROUP_CHUNKS + j4
                        ci = call * chunks_pc + jc
                        oh = opool.tile([128, PSW], F16, tag="oh")
                        nc.vector.tensor_scalar(
                            oh[:], iota_h[:],
                            dr_b[:, ci:ci + 1], dg_b[:, ci:ci + 1],
                            mybir.AluOpType.is_equal, mybir.AluOpType.mult,
                        )
                        nc.tensor.matmul(
                            acc[:], msgs[:, jc, :], oh[:],
                            start=(j4 == 0), stop=(j4 == GROUP_CHUNKS - 1),
                        )
                    # drain: ACT copies psum -> fp16 tmp (ACT can read PSUM),
                    # then DVE adds tmp into agg (all-SBUF fp16 -> fast mode)
                    tmp = tpool.tile([128, PSW], F16, tag="dtmp")
                    nc.scalar.copy(tmp[:], acc[:])
                    dregs = nc.alloc_registers(engines=(mybir.EngineType.DVE,))
                    nc.reg_load(dregs, wbase_sb[0:1, gidx:gidx + 1])
                    sw = nc.snap(dregs, donate=True, min_val=0,
                                 max_val=npad - PSW)
                    nc.vector.tensor_tensor(
                        agg[:, bass.ds(sw, PSW)], tmp[:],
                        agg[:, bass.ds(sw, PSW)], mybir.AluOpType.add,
                    )

            for s in range(n_slabs):
                dop = dpool.tile([128, 512], F32, tag="dop")
                nc.tensor.matmul(dop[:], w_sb[:], agg[:, s * 512:(s + 1) * 512],
                                 start=True, stop=True)
                ot = spool.tile([128, 512], F16, tag="ot")
                nc.scalar.activation(ot[:], dop[:], act_func, bias=b_sb[:, 0:1])
                nc.sync.dma_start(outT[:, s * 512:(s + 1) * 512], ot[:])

    nc.compile()
    return nc


# ----------------------------------------------------------------------------
# host-side graph preprocessing
# ----------------------------------------------------------------------------

def prep_tables(cfg, src, dst):
    """Per-core gather/scatter tables. Returns a list of dicts (one per core)."""
    n_nodes, n_cores, npc = cfg["n_nodes"], cfg["n_cores"], cfg["npc"]
    nbank, bank_sz = cfg["nbank"], cfg["bank_sz"]
    e_bank, call_e, npad = cfg["e_bank"], cfg["call_e"], cfg["npad"]
    chunks_pb = e_bank // CHUNK
    groups_pb = chunks_pb // GROUP_CHUNKS
    n_groups = nbank * groups_pb
    gsz = GROUP_CHUNKS * CHUNK  # edges per group

    deg = np.bincount(dst, minlength=n_nodes)
    deginv = (1.0 / np.maximum(deg, 1)).astype(np.float32)

    core_of = dst // npc
    out = []
    for c in range(n_cores):
        m = core_of == c
        s_c = src[m]
        dl_c = (dst[m] - c * npc).astype(np.int64)
        dg_c = deginv[dst[m]]
        b_c = s_c // bank_sz
        sl_c = (s_c - b_c * bank_sz).astype(np.int16)
        order = np.lexsort((dl_c, b_c))
        s_o, dl_o, dg_o, b_o = sl_c[order], dl_c[order], dg_c[order], b_c[order]

        idx_t = np.zeros((nbank, 128, e_bank // 16), np.int16)
        dr_t = np.full((nbank, 128, chunks_pb), SENT_DSTREL, np.float32)
        dg_t = np.zeros((nbank, 128, chunks_pb), np.float32)
        wbase_t = np.zeros((1, n_groups), np.int32)

        for bb in range(nbank):
            sel = b_o == bb
            sl_b, dl_b, dg_b = s_o[sel], dl_o[sel], dg_o[sel]
            n = len(sl_b)

            # Greedy grouping: groups of up to gsz edges whose dst span
            # stays under PSW; close early (pad with sentinels) otherwise.
            # dl_b is sorted, so group span = last - first.
            starts = [0]
            i = 0
            for j in range(n):
                if j > starts[-1] and (j - starts[-1] >= gsz
                                       or dl_b[j] - dl_b[starts[-1]] >= PSW):
                    starts.append(j)
            starts.append(n)
            n_real_groups = len(starts) - 1 if n > 0 else 0
            assert n_real_groups <= groups_pb, \
                f"core {c} bank {bb}: {n_real_groups} groups > {groups_pb}"

            sl_p = np.zeros(e_bank, np.int16)
            dl_rel = np.full(e_bank, SENT_DSTREL, np.int64)
            dg_p = np.zeros(e_bank, np.float32)
            wb = np.zeros(groups_pb, np.int64)
            for g in range(n_real_groups):
                a, b2 = starts[g], starts[g + 1]
                base = min(int(dl_b[a]), npad - PSW)
                span = int(dl_b[b2 - 1]) - base
                assert 0 <= span < PSW, f"group span {span} >= {PSW}"
                o = g * gsz
                cnt = b2 - a
                sl_p[o:o + cnt] = sl_b[a:b2]
                dl_rel[o:o + cnt] = dl_b[a:b2] - base
                dg_p[o:o + cnt] = dg_b[a:b2]
                wb[g] = base

            dr_t[bb] = dl_rel.reshape(chunks_pb, CHUNK).T.astype(np.float32)
            dg_t[bb] = dg_p.reshape(chunks_pb, CHUNK).T.astype(np.float32)
            wbase_t[0, bb * groups_pb:(bb + 1) * groups_pb] = wb

            # idx wrapping: within each call, idx i -> [16g + i%16, i//16]
            a = sl_p.reshape(e_bank // call_e, call_e // 16, 16)
            blocks = [np.tile(a[k].T, (8, 1)) for k in range(e_bank // call_e)]
            idx_t[bb] = np.concatenate(blocks, axis=1)

        out.append(dict(idx=idx_t, dstrel=dr_t, dege=dg_t, wbase=wbase_t))
    return out


# ----------------------------------------------------------------------------
# full forward
# ----------------------------------------------------------------------------

_compiled = {}


def _get_kernel(cfg, relu=True):
    key = (tuple(sorted(cfg.items())), relu)
    if key not in _compiled:
        _compiled[key] = build_kernel(cfg, relu)
    return _compiled[key]


def run_layer(cfg, nc, tables, x_full, w, b):
    n_cores, npc, npad = cfg["n_cores"], cfg["npc"], cfg["npad"]
    x16 = np.ascontiguousarray(x_full, np.float16)
    w16 = np.ascontiguousarray(w, np.float16)
    bcol = np.ascontiguousarray(b, np.float32).reshape(D, 1)
    in_maps = []
    for c in range(n_cores):
        xT = np.zeros((D, npad), np.float16)
        xT[:, :npc] = x16[c * npc:(c + 1) * npc].T
        t = tables[c]
        in_maps.append({
            "xfull": x16, "xT": xT, "w": w16, "bcol": bcol,
            "idx": t["idx"], "dstrel": t["dstrel"], "dege": t["dege"],
            "wbase": t["wbase"],
        })
    res = run_bass_kernel_spmd(nc, in_maps, core_ids=list(range(n_cores)))
    out = np.empty((n_cores * npc, D), np.float32)
    for c in range(n_cores):
        out[c * npc:(c + 1) * npc] = res.results[c]["outT"][:, :npc].T
    return out


def gin_forward(cfg, in_feat, src, dst, W1, b1, W2, b2):
    nc1 = _get_kernel(cfg, relu=True)
    nc2 = _get_kernel(cfg, relu=False)
    tables = prep_tables(cfg, src, dst)
    x = np.ascontiguousarray(in_feat, np.float32)
    h = run_layer(cfg, nc1, tables, x, W1, b1)
    return run_layer(cfg, nc2, tables, h, W2, b2)


def kernel(in_feat, src, dst, W1, b1, W2, b2):
    in_feat = np.asarray(in_feat, np.float32)
    src = np.asarray(src, np.int64)
    dst = np.asarray(dst, np.int64)
    W1 = np.asarray(W1, np.float32)
    b1 = np.asarray(b1, np.float32)
    W2 = np.asarray(W2, np.float32)
    b2 = np.asarray(b2, np.float32)
    return gin_forward(FULL_CFG, in_feat, src, dst, W1, b1, W2, b2)


# revision 9
# speedup vs baseline: 1.1521x; 1.0669x over previous
"""2-layer GIN (mean aggregation) on 8 Trainium2 NeuronCores.

Strategy (graph/data parallel, per the node-sharding hint):
  - Nodes sharded 8 ways by dst; each core owns its node shard and all
    edges whose dst lands in the shard.
  - Per layer, per core: agg_T[f, d] = x_T + sum_e msg[src(e)] * (1/deg),
    then the dense epilogue out_T = W.T @ agg_T + b (+relu), feature-major.
  - Edge messages are fetched with gpsimd.dma_gather (SWDGE) straight from
    an HBM copy of the full feature table (fp16). int16 gather indices cap
    a table at 32768 rows, so the node table is split into 4 banks of
    25000 and each core's edge list is sorted by (bank, dst).
  - Segment-sum: per 128-edge chunk, DVE builds a [128 e, 192 d] fp16
    one-hot = (iota == dstrel) * deginv against host-baked window-relative
    dst offsets; PE accumulates psum[f, 0:192] += msgs[e, f].T @ onehot
    over a 4-chunk (512-edge) group; the group psum is drain-added into
    the fp16 SBUF agg accumulator at a register-driven (data-dependent
    window base) offset, alternating between the DVE and Pool engines so
    neither becomes the bottleneck. The agg tile is DMA-initialized with
    x_T so the (1+eps)*x term needs no extra pass.
  - Gather calls are 5120 edges each (large calls amortize the SWDGE
    fixed descriptor-generation overhead on the Pool engine) and are
    emitted one call ahead of their compute so the DMA engines stay busy.
  - Dense epilogue: PE matmul (fp16) per 512-column slab, then the
    Activation engine applies bias+relu (or bias only) straight out of
    PSUM into an fp16 staging tile that is DMA'd out.
  - The two GIN layers are two executions of near-identical NEFFs (they
    differ only in relu-vs-identity); the host concatenates the per-core
    shard outputs between layers (the halo exchange runs through host
    memory). The SPMD instruction stream is identical across cores —
    only table data varies.
"""

import numpy as np

import concourse.bass as bass
import concourse.mybir as mybir
import concourse.tile as tile
from concourse import bacc
from concourse.bass_utils import run_bass_kernel_spmd

F32 = mybir.dt.float32
F16 = mybir.dt.float16
I32 = mybir.dt.int32
I16 = mybir.dt.int16

FULL_CFG = dict(
    n_nodes=100000,   # global nodes (gather table rows)
    n_cores=8,
    npc=12500,        # nodes per core
    npad=12800,       # padded nodes per core (multiple of 512)
    nbank=4,
    bank_sz=25000,
    e_bank=51200,     # padded edges per (core, bank); multiple of call_e
    call_e=1024,      # edges per dma_gather call (hard SWDGE per-call cap)
)

D = 128
CHUNK = 128
GROUP_CHUNKS = 4   # 512-edge psum accumulation group
PSW = 144          # psum window width (dst columns per group)
SENT_DSTREL = PSW - 1   # sentinel: in-window column, dege=0 -> contributes 0


# ----------------------------------------------------------------------------
# device kernel
# ----------------------------------------------------------------------------

def build_kernel(cfg, relu):
    nbank, bank_sz = cfg["nbank"], cfg["bank_sz"]
    e_bank, call_e, npad = cfg["e_bank"], cfg["call_e"], cfg["npad"]
    calls_pb = e_bank // call_e
    chunks_pc = call_e // CHUNK
    groups_pc = chunks_pc // GROUP_CHUNKS
    chunks_pb = e_bank // CHUNK
    groups_pb = chunks_pb // GROUP_CHUNKS
    n_groups = nbank * groups_pb
    n_slabs = npad // 512

    nc = bacc.Bacc("TRN2", target_bir_lowering=False, debug=False,
                   num_devices=cfg["n_cores"],
                   dynamic_dma_scratch_size=98304)

    xfull = nc.dram_tensor("xfull", [cfg["n_nodes"], D], F16, kind="ExternalInput")
    xT = nc.dram_tensor("xT", [D, npad], F16, kind="ExternalInput")
    w = nc.dram_tensor("w", [D, D], F16, kind="ExternalInput")
    bcol = nc.dram_tensor("bcol", [D, 1], F32, kind="ExternalInput")
    idx = nc.dram_tensor("idx", [nbank, 128, e_bank // 16], I16, kind="ExternalInput")
    dstrel = nc.dram_tensor("dstrel", [nbank, 128, chunks_pb], F32, kind="ExternalInput")
    dege = nc.dram_tensor("dege", [nbank, 128, chunks_pb], F32, kind="ExternalInput")
    wbase = nc.dram_tensor("wbase", [1, n_groups], I32, kind="ExternalInput")
    outT = nc.dram_tensor("outT", [D, npad], F16, kind="ExternalOutput")

    act_func = (mybir.ActivationFunctionType.Relu if relu
                else mybir.ActivationFunctionType.Identity)

    with tile.TileContext(nc) as tc:
        with (
            tc.tile_pool(name="const", bufs=1) as cpool,
            tc.tile_pool(name="agg", bufs=1) as apool,
            tc.tile_pool(name="btab", bufs=2) as bpool,
            tc.tile_pool(name="msgs", bufs=3) as mpool,
            tc.tile_pool(name="oh", bufs=4) as opool,
            tc.tile_pool(name="dtmp", bufs=3) as tpool,
            tc.tile_pool(name="acc", bufs=3, space="PSUM") as pspool,
            tc.tile_pool(name="dense", bufs=3, space="PSUM") as dpool,
            tc.tile_pool(name="osb", bufs=4) as spool,
        ):
            iota_i = cpool.tile([128, PSW], I32)
            nc.gpsimd.iota(iota_i[:], pattern=[[1, PSW]], channel_multiplier=0)
            iota_h = cpool.tile([128, PSW], F16)
            nc.vector.tensor_copy(iota_h[:], iota_i[:])

            # (bank, call) schedule, with table loads and gathers emitted one
            # call ahead of their compute so desc-gen/DMA overlap compute.
            sched = [(bb, call) for bb in range(nbank) for call in range(calls_pb)]
            btabs = {}
            mtiles = [None] * len(sched)

            def load_bank(bb):
                idx_b = bpool.tile([128, e_bank // 16], I16, tag="idx")
                nc.sync.dma_start(idx_b[:], idx[bb])
                dr_b = bpool.tile([128, chunks_pb], F32, tag="dr")
                nc.sync.dma_start(dr_b[:], dstrel[bb])
                dg_b = bpool.tile([128, chunks_pb], F32, tag="dg")
                nc.sync.dma_start(dg_b[:], dege[bb])
                btabs[bb] = (idx_b, dr_b, dg_b)

            def emit_gather(k):
                bb, call = sched[k]
                if call == 0:
                    load_bank(bb)
                idx_b = btabs[bb][0]
                msgs = mpool.tile([128, chunks_pc, D], F16, tag="msgs")
                nc.gpsimd.dma_gather(
                    msgs[:], xfull[bb * bank_sz:(bb + 1) * bank_sz, :],
                    idx_b[:, call * (call_e // 16):(call + 1) * (call_e // 16)],
                    call_e, call_e, D,
                )
                mtiles[k] = msgs

            emit_gather(0)

            # constants + agg init are emitted after the first gather so the
            # gather's idx-table DMA isn't queued behind them.
            wbase_sb = cpool.tile([1, n_groups], I32)
            nc.sync.dma_start(wbase_sb[:], wbase[:])
            w_sb = cpool.tile([D, D], F16)
            nc.sync.dma_start(w_sb[:], w[:])
            b_sb = cpool.tile([D, 1], F32)
            nc.sync.dma_start(b_sb[:], bcol[:])
            # agg starts as x_T; group psums are drain-added into it.
            agg = apool.tile([128, npad], F16)
            nc.sync.dma_start(agg[:], xT[:])

            for k, (bb, call) in enumerate(sched):
                if k + 1 < len(sched):
                    emit_gather(k + 1)
                msgs = mtiles[k]
                _, dr_b, dg_b = btabs[bb]
                for g2 in range(groups_pc):
                    gidx = bb * groups_pb + call * groups_pc + g2
                    acc = pspool.tile([128, PSW], F32, tag="acc")
                    for j4 in range(GROUP_CHUNKS):
                        jc = g2 * GROUP_CHUNKS + j4
                        ci = call * chunks_pc + jc
                        oh = opool.tile([128, PSW], F16, tag="oh")
                        nc.vector.tensor_scalar(
                            oh[:], iota_h[:],
                            dr_b[:, ci:ci + 1], dg_b[:, ci:ci + 1],
                            mybir.AluOpType.is_equal, mybir.AluOpType.mult,
                        )
                        nc.tensor.matmul(
                            acc[:], msgs[:, jc, :], oh[:],
                            start=(j4 == 0), stop=(j4 == GROUP_CHUNKS - 1),
                        )
                    # drain: ACT copies psum -> fp16 tmp (ACT can read PSUM),
                    # then DVE adds tmp into agg (all-SBUF fp16 -> fast mode)
                    tmp = tpool.tile([128, PSW], F16, tag="dtmp")
                    nc.scalar.copy(tmp[:], acc[:])
                    dregs = nc.alloc_registers(engines=(mybir.EngineType.DVE,))
                    nc.reg_load(dregs, wbase_sb[0:1, gidx:gidx + 1])
                    sw = nc.snap(dregs, donate=True, min_val=0,
                                 max_val=npad - PSW)
                    nc.vector.tensor_tensor(
                        agg[:, bass.ds(sw, PSW)], tmp[:],
                        agg[:, bass.ds(sw, PSW)], mybir.AluOpType.add,
                    )

            relu_lo = 0.0 if relu else -3.0e38
            for s in range(n_slabs):
                dop = dpool.tile([128, 512], F32, tag="dop")
                nc.tensor.matmul(dop[:], w_sb[:], agg[:, s * 512:(s + 1) * 512],
                                 start=True, stop=True)
                ot = spool.tile([128, 512], F16, tag="ot")
                if s % 2 == 0:
                    nc.scalar.activation(ot[:], dop[:], act_func,
                                         bias=b_sb[:, 0:1])
                else:
                    nc.vector.tensor_scalar(
                        ot[:], dop[:], b_sb[:, 0:1], relu_lo,
                        mybir.AluOpType.add, mybir.AluOpType.max,
                    )
                nc.sync.dma_start(outT[:, s * 512:(s + 1) * 512], ot[:])

    nc.compile()
    return nc


# ----------------------------------------------------------------------------
# host-side graph preprocessing
# ----------------------------------------------------------------------------

def prep_tables(cfg, src, dst):
    """Per-core gather/scatter tables. Returns a list of dicts (one per core)."""
    n_nodes, n_cores, npc = cfg["n_nodes"], cfg["n_cores"], cfg["npc"]
    nbank, bank_sz = cfg["nbank"], cfg["bank_sz"]
    e_bank, call_e, npad = cfg["e_bank"], cfg["call_e"], cfg["npad"]
    chunks_pb = e_bank // CHUNK
    groups_pb = chunks_pb // GROUP_CHUNKS
    n_groups = nbank * groups_pb
    gsz = GROUP_CHUNKS * CHUNK  # edges per group

    deg = np.bincount(dst, minlength=n_nodes)
    deginv = (1.0 / np.maximum(deg, 1)).astype(np.float32)

    core_of = dst // npc
    out = []
    for c in range(n_cores):
        m = core_of == c
        s_c = src[m]
        dl_c = (dst[m] - c * npc).astype(np.int64)
        dg_c = deginv[dst[m]]
        b_c = s_c // bank_sz
        sl_c = (s_c - b_c * bank_sz).astype(np.int16)
        order = np.lexsort((dl_c, b_c))
        s_o, dl_o, dg_o, b_o = sl_c[order], dl_c[order], dg_c[order], b_c[order]

        idx_t = np.zeros((nbank, 128, e_bank // 16), np.int16)
        dr_t = np.full((nbank, 128, chunks_pb), SENT_DSTREL, np.float32)
        dg_t = np.zeros((nbank, 128, chunks_pb), np.float32)
        wbase_t = np.zeros((1, n_groups), np.int32)

        for bb in range(nbank):
            sel = b_o == bb
            sl_b, dl_b, dg_b = s_o[sel], dl_o[sel], dg_o[sel]
            n = len(sl_b)

            # Greedy grouping: groups of up to gsz edges whose dst span
            # stays under PSW; close early (pad with sentinels) otherwise.
            # dl_b is sorted, so group span = last - first.
            starts = [0]
            i = 0
            for j in range(n):
                if j > starts[-1] and (j - starts[-1] >= gsz
                                       or dl_b[j] - dl_b[starts[-1]] >= PSW):
                    starts.append(j)
            starts.append(n)
            n_real_groups = len(starts) - 1 if n > 0 else 0
            assert n_real_groups <= groups_pb, \
                f"core {c} bank {bb}: {n_real_groups} groups > {groups_pb}"

            sl_p = np.zeros(e_bank, np.int16)
            dl_rel = np.full(e_bank, SENT_DSTREL, np.int64)
            dg_p = np.zeros(e_bank, np.float32)
            wb = np.zeros(groups_pb, np.int64)
            for g in range(n_real_groups):
                a, b2 = starts[g], starts[g + 1]
                base = min(int(dl_b[a]), npad - PSW)
                span = int(dl_b[b2 - 1]) - base
                assert 0 <= span < PSW, f"group span {span} >= {PSW}"
                o = g * gsz
                cnt = b2 - a
                sl_p[o:o + cnt] = sl_b[a:b2]
                dl_rel[o:o + cnt] = dl_b[a:b2] - base
                dg_p[o:o + cnt] = dg_b[a:b2]
                wb[g] = base

            dr_t[bb] = dl_rel.reshape(chunks_pb, CHUNK).T.astype(np.float32)
            dg_t[bb] = dg_p.reshape(chunks_pb, CHUNK).T.astype(np.float32)
            wbase_t[0, bb * groups_pb:(bb + 1) * groups_pb] = wb

            # idx wrapping: within each call, idx i -> [16g + i%16, i//16]
            a = sl_p.reshape(e_bank // call_e, call_e // 16, 16)
            blocks = [np.tile(a[k].T, (8, 1)) for k in range(e_bank // call_e)]
            idx_t[bb] = np.concatenate(blocks, axis=1)

        out.append(dict(idx=idx_t, dstrel=dr_t, dege=dg_t, wbase=wbase_t))
    return out


# ----------------------------------------------------------------------------
# full forward
# ----------------------------------------------------------------------------

_compiled = {}


def _get_kernel(cfg, relu=True):
    key = (tuple(sorted(cfg.items())), relu)
    if key not in _compiled:
        _compiled[key] = build_kernel(cfg, relu)
    return _compiled[key]


def run_layer(cfg, nc, tables, x_full, w, b):
    n_cores, npc, npad = cfg["n_cores"], cfg["npc"], cfg["npad"]
    x16 = np.ascontiguousarray(x_full, np.float16)
    w16 = np.ascontiguousarray(w, np.float16)
    bcol = np.ascontiguousarray(b, np.float32).reshape(D, 1)
    in_maps = []
    for c in range(n_cores):
        xT = np.zeros((D, npad), np.float16)
        xT[:, :npc] = x16[c * npc:(c + 1) * npc].T
        t = tables[c]
        in_maps.append({
            "xfull": x16, "xT": xT, "w": w16, "bcol": bcol,
            "idx": t["idx"], "dstrel": t["dstrel"], "dege": t["dege"],
            "wbase": t["wbase"],
        })
    res = run_bass_kernel_spmd(nc, in_maps, core_ids=list(range(n_cores)))
    out = np.empty((n_cores * npc, D), np.float32)
    for c in range(n_cores):
        out[c * npc:(c + 1) * npc] = res.results[c]["outT"][:, :npc].T
    return out


def gin_forward(cfg, in_feat, src, dst, W1, b1, W2, b2):
    nc1 = _get_kernel(cfg, relu=True)
    nc2 = _get_kernel(cfg, relu=False)
    tables = prep_tables(cfg, src, dst)
    x = np.ascontiguousarray(in_feat, np.float32)
    h = run_layer(cfg, nc1, tables, x, W1, b1)
    return run_layer(cfg, nc2, tables, h, W2, b2)


def kernel(in_feat, src, dst, W1, b1, W2, b2):
    in_feat = np.asarray(in_feat, np.float32)
    src = np.asarray(src, np.int64)
    dst = np.asarray(dst, np.int64)
    W1 = np.asarray(W1, np.float32)
    b1 = np.asarray(b1, np.float32)
    W2 = np.asarray(W2, np.float32)
    b2 = np.asarray(b2, np.float32)
    return gin_forward(FULL_CFG, in_feat, src, dst, W1, b1, W2, b2)


# revision 10
# speedup vs baseline: 1.2432x; 1.0791x over previous
"""2-layer GIN (mean aggregation) on 8 Trainium2 NeuronCores.

Strategy (graph/data parallel, per the node-sharding hint):
  - Nodes sharded 8 ways by dst; each core owns its node shard and all
    edges whose dst lands in the shard.
  - Per layer, per core: agg_T[f, d] = x_T + sum_e msg[src(e)] * (1/deg),
    then the dense epilogue out_T = W.T @ agg_T + b (+relu), feature-major.
  - Edge messages are fetched with gpsimd.dma_gather (SWDGE) straight from
    an HBM copy of the full feature table (fp16). int16 gather indices cap
    a table at 32768 rows, so the node table is split into 4 banks of
    25000 and each core's edge list is sorted by (bank, dst).
  - Segment-sum: per 128-edge chunk, DVE builds a [128 e, 192 d] fp16
    one-hot = (iota == dstrel) * deginv against host-baked window-relative
    dst offsets; PE accumulates psum[f, 0:192] += msgs[e, f].T @ onehot
    over a 4-chunk (512-edge) group; the group psum is drain-added into
    the fp16 SBUF agg accumulator at a register-driven (data-dependent
    window base) offset, alternating between the DVE and Pool engines so
    neither becomes the bottleneck. The agg tile is DMA-initialized with
    x_T so the (1+eps)*x term needs no extra pass.
  - Gather calls are 5120 edges each (large calls amortize the SWDGE
    fixed descriptor-generation overhead on the Pool engine) and are
    emitted one call ahead of their compute so the DMA engines stay busy.
  - Dense epilogue: PE matmul (fp16) per 512-column slab, then the
    Activation engine applies bias+relu (or bias only) straight out of
    PSUM into an fp16 staging tile that is DMA'd out.
  - The two GIN layers are two executions of near-identical NEFFs (they
    differ only in relu-vs-identity); the host concatenates the per-core
    shard outputs between layers (the halo exchange runs through host
    memory). The SPMD instruction stream is identical across cores —
    only table data varies.
"""

import numpy as np

import concourse.bass as bass
import concourse.mybir as mybir
import concourse.tile as tile
from concourse import bacc
from concourse.bass_utils import run_bass_kernel_spmd

F32 = mybir.dt.float32
F16 = mybir.dt.float16
I32 = mybir.dt.int32
I16 = mybir.dt.int16

FULL_CFG = dict(
    n_nodes=100000,   # global nodes (gather table rows)
    n_cores=8,
    npc=12500,        # nodes per core
    npad=12800,       # padded nodes per core (multiple of 512)
    nbank=4,
    bank_sz=25000,
    e_bank=51200,     # padded edges per (core, bank); multiple of call_e
    call_e=1024,      # edges per dma_gather call (hard SWDGE per-call cap)
)

D = 128
CHUNK = 128
GROUP_CHUNKS = 4   # 512-edge psum accumulation group
PSW = 144          # psum window width (dst columns per group)
SENT_DSTREL = PSW - 1   # sentinel: in-window column, dege=0 -> contributes 0


# ----------------------------------------------------------------------------
# device kernel
# ----------------------------------------------------------------------------

def build_kernel(cfg, relu):
    nbank, bank_sz = cfg["nbank"], cfg["bank_sz"]
    e_bank, call_e, npad = cfg["e_bank"], cfg["call_e"], cfg["npad"]
    calls_pb = e_bank // call_e
    chunks_pc = call_e // CHUNK
    groups_pc = chunks_pc // GROUP_CHUNKS
    chunks_pb = e_bank // CHUNK
    groups_pb = chunks_pb // GROUP_CHUNKS
    n_groups = nbank * groups_pb
    n_slabs = npad // 512

    nc = bacc.Bacc("TRN2", target_bir_lowering=False, debug=False,
                   num_devices=cfg["n_cores"],
                   dynamic_dma_scratch_size=98304)

    xfull = nc.dram_tensor("xfull", [cfg["n_nodes"], D], F16, kind="ExternalInput")
    xT = nc.dram_tensor("xT", [D, npad], F16, kind="ExternalInput")
    w = nc.dram_tensor("w", [D, D], F16, kind="ExternalInput")
    bcol = nc.dram_tensor("bcol", [D, 1], F32, kind="ExternalInput")
    idx = nc.dram_tensor("idx", [nbank, 128, e_bank // 16], I16, kind="ExternalInput")
    dstrel = nc.dram_tensor("dstrel", [nbank, 128, chunks_pb], F32, kind="ExternalInput")
    dege = nc.dram_tensor("dege", [nbank, 128, chunks_pb], F32, kind="ExternalInput")
    wbase = nc.dram_tensor("wbase", [1, n_groups], I32, kind="ExternalInput")
    outT = nc.dram_tensor("outT", [D, npad], F16, kind="ExternalOutput")

    act_func = (mybir.ActivationFunctionType.Relu if relu
                else mybir.ActivationFunctionType.Identity)

    with tile.TileContext(nc) as tc:
        with (
            tc.tile_pool(name="const", bufs=1) as cpool,
            tc.tile_pool(name="agg", bufs=1) as apool,
            tc.tile_pool(name="btab", bufs=2) as bpool,
            tc.tile_pool(name="msgs", bufs=4) as mpool,
            tc.tile_pool(name="oh", bufs=4) as opool,
            tc.tile_pool(name="dtmp", bufs=3) as tpool,
            tc.tile_pool(name="acc", bufs=4, space="PSUM") as pspool,
            tc.tile_pool(name="dense", bufs=3, space="PSUM") as dpool,
            tc.tile_pool(name="osb", bufs=4) as spool,
        ):
            iota_i = cpool.tile([128, PSW], I32)
            nc.gpsimd.iota(iota_i[:], pattern=[[1, PSW]], channel_multiplier=0)
            iota_h = cpool.tile([128, PSW], F16)
            nc.vector.tensor_copy(iota_h[:], iota_i[:])

            # (bank, call) schedule, with table loads and gathers emitted one
            # call ahead of their compute so desc-gen/DMA overlap compute.
            sched = [(bb, call) for bb in range(nbank) for call in range(calls_pb)]
            btabs = {}
            mtiles = [None] * len(sched)

            def load_bank(bb):
                idx_b = bpool.tile([128, e_bank // 16], I16, tag="idx")
                nc.sync.dma_start(idx_b[:], idx[bb])
                dr_b = bpool.tile([128, chunks_pb], F32, tag="dr")
                nc.sync.dma_start(dr_b[:], dstrel[bb])
                dg_b = bpool.tile([128, chunks_pb], F32, tag="dg")
                nc.sync.dma_start(dg_b[:], dege[bb])
                btabs[bb] = (idx_b, dr_b, dg_b)

            def emit_gather(k):
                bb, call = sched[k]
                if k == 0:
                    load_bank(bb)
                # prefetch the next bank's tables a few calls early so the
                # idx DMA lands before the first gather of that bank
                if call == calls_pb - 3 and bb + 1 < nbank:
                    load_bank(bb + 1)
                idx_b = btabs[bb][0]
                msgs = mpool.tile([128, chunks_pc, D], F16, tag="msgs")
                nc.gpsimd.dma_gather(
                    msgs[:], xfull[bb * bank_sz:(bb + 1) * bank_sz, :],
                    idx_b[:, call * (call_e // 16):(call + 1) * (call_e // 16)],
                    call_e, call_e, D,
                )
                mtiles[k] = msgs

            emit_gather(0)

            # constants + agg init are emitted after the first gather so the
            # gather's idx-table DMA isn't queued behind them.
            wbase_sb = cpool.tile([1, n_groups], I32)
            nc.sync.dma_start(wbase_sb[:], wbase[:])
            w_sb = cpool.tile([D, D], F16)
            nc.sync.dma_start(w_sb[:], w[:])
            b_sb = cpool.tile([D, 1], F32)
            nc.sync.dma_start(b_sb[:], bcol[:])
            # agg starts as x_T; group psums are drain-added into it.
            agg = apool.tile([128, npad], F16)
            nc.sync.dma_start(agg[:], xT[:])

            for k, (bb, call) in enumerate(sched):
                if k + 1 < len(sched):
                    emit_gather(k + 1)
                msgs = mtiles[k]
                _, dr_b, dg_b = btabs[bb]
                for g2 in range(groups_pc):
                    gidx = bb * groups_pb + call * groups_pc + g2
                    acc = pspool.tile([128, PSW], F32, tag="acc")
                    for j4 in range(GROUP_CHUNKS):
                        jc = g2 * GROUP_CHUNKS + j4
                        ci = call * chunks_pc + jc
                        oh = opool.tile([128, PSW], F16, tag="oh")
                        nc.vector.tensor_scalar(
                            oh[:], iota_h[:],
                            dr_b[:, ci:ci + 1], dg_b[:, ci:ci + 1],
                            mybir.AluOpType.is_equal, mybir.AluOpType.mult,
                        )
                        nc.tensor.matmul(
                            acc[:], msgs[:, jc, :], oh[:],
                            start=(j4 == 0), stop=(j4 == GROUP_CHUNKS - 1),
                        )
                    # drain: ACT copies psum -> fp16 tmp (ACT can read PSUM),
                    # then DVE adds tmp into agg (all-SBUF fp16 -> fast mode)
                    tmp = tpool.tile([128, PSW], F16, tag="dtmp")
                    nc.scalar.copy(tmp[:], acc[:])
                    dregs = nc.alloc_registers(engines=(mybir.EngineType.DVE,))
                    nc.reg_load(dregs, wbase_sb[0:1, gidx:gidx + 1])
                    sw = nc.snap(dregs, donate=True, min_val=0,
                                 max_val=npad - PSW)
                    nc.vector.tensor_tensor(
                        agg[:, bass.ds(sw, PSW)], tmp[:],
                        agg[:, bass.ds(sw, PSW)], mybir.AluOpType.add,
                    )

            relu_lo = 0.0 if relu else -3.0e38
            for s in range(n_slabs):
                dop = dpool.tile([128, 512], F32, tag="dop")
                nc.tensor.matmul(dop[:], w_sb[:], agg[:, s * 512:(s + 1) * 512],
                                 start=True, stop=True)
                ot = spool.tile([128, 512], F16, tag="ot")
                if s % 2 == 0:
                    nc.scalar.activation(ot[:], dop[:], act_func,
                                         bias=b_sb[:, 0:1])
                else:
                    nc.vector.tensor_scalar(
                        ot[:], dop[:], b_sb[:, 0:1], relu_lo,
                        mybir.AluOpType.add, mybir.AluOpType.max,
                    )
                nc.sync.dma_start(outT[:, s * 512:(s + 1) * 512], ot[:])

    nc.compile()
    return nc


# ----------------------------------------------------------------------------
# host-side graph preprocessing
# ----------------------------------------------------------------------------

def prep_tables(cfg, src, dst):
    """Per-core gather/scatter tables. Returns a list of dicts (one per core)."""
    n_nodes, n_cores, npc = cfg["n_nodes"], cfg["n_cores"], cfg["npc"]
    nbank, bank_sz = cfg["nbank"], cfg["bank_sz"]
    e_bank, call_e, npad = cfg["e_bank"], cfg["call_e"], cfg["npad"]
    chunks_pb = e_bank // CHUNK
    groups_pb = chunks_pb // GROUP_CHUNKS
    n_groups = nbank * groups_pb
    gsz = GROUP_CHUNKS * CHUNK  # edges per group

    deg = np.bincount(dst, minlength=n_nodes)
    deginv = (1.0 / np.maximum(deg, 1)).astype(np.float32)

    core_of = dst // npc
    out = []
    for c in range(n_cores):
        m = core_of == c
        s_c = src[m]
        dl_c = (dst[m] - c * npc).astype(np.int64)
        dg_c = deginv[dst[m]]
        b_c = s_c // bank_sz
        sl_c = (s_c - b_c * bank_sz).astype(np.int16)
        order = np.lexsort((dl_c, b_c))
        s_o, dl_o, dg_o, b_o = sl_c[order], dl_c[order], dg_c[order], b_c[order]

        idx_t = np.zeros((nbank, 128, e_bank // 16), np.int16)
        dr_t = np.full((nbank, 128, chunks_pb), SENT_DSTREL, np.float32)
        dg_t = np.zeros((nbank, 128, chunks_pb), np.float32)
        wbase_t = np.zeros((1, n_groups), np.int32)

        for bb in range(nbank):
            sel = b_o == bb
            sl_b, dl_b, dg_b = s_o[sel], dl_o[sel], dg_o[sel]
            n = len(sl_b)

            # Greedy grouping: groups of up to gsz edges whose dst span
            # stays under PSW; close early (pad with sentinels) otherwise.
            # dl_b is sorted, so group span = last - first.
            starts = [0]
            i = 0
            for j in range(n):
                if j > starts[-1] and (j - starts[-1] >= gsz
                                       or dl_b[j] - dl_b[starts[-1]] >= PSW):
                    starts.append(j)
            starts.append(n)
            n_real_groups = len(starts) - 1 if n > 0 else 0
            assert n_real_groups <= groups_pb, \
                f"core {c} bank {bb}: {n_real_groups} groups > {groups_pb}"

            sl_p = np.zeros(e_bank, np.int16)
            dl_rel = np.full(e_bank, SENT_DSTREL, np.int64)
            dg_p = np.zeros(e_bank, np.float32)
            wb = np.zeros(groups_pb, np.int64)
            for g in range(n_real_groups):
                a, b2 = starts[g], starts[g + 1]
                base = min(int(dl_b[a]), npad - PSW)
                span = int(dl_b[b2 - 1]) - base
                assert 0 <= span < PSW, f"group span {span} >= {PSW}"
                o = g * gsz
                cnt = b2 - a
                sl_p[o:o + cnt] = sl_b[a:b2]
                dl_rel[o:o + cnt] = dl_b[a:b2] - base
                dg_p[o:o + cnt] = dg_b[a:b2]
                wb[g] = base

            dr_t[bb] = dl_rel.reshape(chunks_pb, CHUNK).T.astype(np.float32)
            dg_t[bb] = dg_p.reshape(chunks_pb, CHUNK).T.astype(np.float32)
            wbase_t[0, bb * groups_pb:(bb + 1) * groups_pb] = wb

            # idx wrapping: within each call, idx i -> [16g + i%16, i//16]
            a = sl_p.reshape(e_bank // call_e, call_e // 16, 16)
            blocks = [np.tile(a[k].T, (8, 1)) for k in range(e_bank // call_e)]
            idx_t[bb] = np.concatenate(blocks, axis=1)

        out.append(dict(idx=idx_t, dstrel=dr_t, dege=dg_t, wbase=wbase_t))
    return out


# ----------------------------------------------------------------------------
# full forward
# ----------------------------------------------------------------------------

_compiled = {}


def _get_kernel(cfg, relu=True):
    key = (tuple(sorted(cfg.items())), relu)
    if key not in _compiled:
        _compiled[key] = build_kernel(cfg, relu)
    return _compiled[key]


def run_layer(cfg, nc, tables, x_full, w, b):
    n_cores, npc, npad = cfg["n_cores"], cfg["npc"], cfg["npad"]
    x16 = np.ascontiguousarray(x_full, np.float16)
    w16 = np.ascontiguousarray(w, np.float16)
    bcol = np.ascontiguousarray(b, np.float32).reshape(D, 1)
    in_maps = []
    for c in range(n_cores):
        xT = np.zeros((D, npad), np.float16)
        xT[:, :npc] = x16[c * npc:(c + 1) * npc].T
        t = tables[c]
        in_maps.append({
            "xfull": x16, "xT": xT, "w": w16, "bcol": bcol,
            "idx": t["idx"], "dstrel": t["dstrel"], "dege": t["dege"],
            "wbase": t["wbase"],
        })
    res = run_bass_kernel_spmd(nc, in_maps, core_ids=list(range(n_cores)))
    out = np.empty((n_cores * npc, D), np.float32)
    for c in range(n_cores):
        out[c * npc:(c + 1) * npc] = res.results[c]["outT"][:, :npc].T
    return out


def gin_forward(cfg, in_feat, src, dst, W1, b1, W2, b2):
    nc1 = _get_kernel(cfg, relu=True)
    nc2 = _get_kernel(cfg, relu=False)
    tables = prep_tables(cfg, src, dst)
    x = np.ascontiguousarray(in_feat, np.float32)
    h = run_layer(cfg, nc1, tables, x, W1, b1)
    return run_layer(cfg, nc2, tables, h, W2, b2)


def kernel(in_feat, src, dst, W1, b1, W2, b2):
    in_feat = np.asarray(in_feat, np.float32)
    src = np.asarray(src, np.int64)
    dst = np.asarray(dst, np.int64)
    W1 = np.asarray(W1, np.float32)
    b1 = np.asarray(b1, np.float32)
    W2 = np.asarray(W2, np.float32)
    b2 = np.asarray(b2, np.float32)
    return gin_forward(FULL_CFG, in_feat, src, dst, W1, b1, W2, b2)
